# revision 8
# baseline (speedup 1.0000x reference)
"""Trainium2 Bass kernel for nn_NeuralODE: 11 Euler steps of
    z += tanh([z, t0e, t1e] @ W1 + b1) @ W2 + b2
with z=[128, 17728], W1=[17740, 4096], W2=[4096, 17728], on 8 NeuronCores.

Strategy: tensor-parallel over the Z dim (17728 = 8 x 2216 per core).
Each core holds z columns [c*2216, (c+1)*2216) and computes a partial
pre-activation  pre_c = z_c @ W1[rows_c, :]  ([128, 4096]), which is
AllReduced across cores (bf16, two 2048-wide halves so the collective
overlaps compute).  The partial pre is TRANSPOSED on-core before the
AllReduce (in the compute-covered window), so the summed result comes back
as pre^T and tanh directly yields h^T blocks — nothing but DMA+tanh sits on
the critical AllReduce -> mm2 path.  dz_c = h @ W2[:, cols_c] is fully
local.  W2's column shard stays resident in SBUF (bf16); W1's row shard
streams from HBM every step.  Time-embedding + b1 enter as 13 extra
contraction rows (scaled by 1/8 so the AllReduce sums back to x1); b2
enters mm2 as a K=1 matmul with a ones vector.
"""

import os
import sys

for _p in ("/opt/trn_rl_repo", "/root/.axon_site/_ro/trn_rl_repo"):
    if os.path.isdir(_p) and _p not in sys.path:
        sys.path.append(_p)

import numpy as np

import concourse.bacc as bacc
import concourse.bass as bass
import concourse.mybir as mybir
from concourse import masks, tile
from concourse.bass_utils import run_bass_kernel_spmd

# ---------------------------------------------------------------- constants
N_CORES = 8
CORE_IDS = list(range(N_CORES))
BS = 128                      # batch
H, W, L, D = 56, 56, 64, 128
Z = 3 * H * W + L * D + D     # 17728
HID = 4096
FREQ = 3
MAX_STEP = 4
NSTEPS = MAX_STEP * FREQ - 1  # 11
PERIODS = np.array([24.0, 7.0, 365.0], dtype=np.float32)
INFS = np.array([0.0, 0.0, 0.0], dtype=np.float32)

SHARD = Z // N_CORES          # 2216
NKZ = 18                      # z k-tiles per core (17 full + 1 partial, padded)
ZP = NKZ * 128                # 2304 padded shard
NH = HID // 128               # 32 hid k-tiles
HHID = HID // 2               # 2048: AllReduce half
NHH = HHID // 128             # 16 hid k-tiles per half
# dz N-chunks over the 2216 shard
DZ_CHUNKS = [(0, 512), (512, 512), (1024, 512), (1536, 512), (2048, 168)]
CROWS = 2 * FREQ * 2 + 1      # 13 = t0e(6) + t1e(6) + bias row

F32 = mybir.dt.float32
BF16 = mybir.dt.bfloat16
NP_BF16 = mybir.dt.np(BF16)

_CACHED_NC = None


def build_nc():
    """Build + compile the SPMD program (identical on all 8 cores)."""
    nc = bacc.Bacc(None, num_devices=N_CORES)

    w1c_ext = nc.dram_tensor("w1c", [ZP, HID], BF16, kind="ExternalInput")
    w2c_ext = nc.dram_tensor("w2c", [HID, SHARD], BF16, kind="ExternalInput")
    z0t_ext = nc.dram_tensor("z0t", [128, ZP], BF16, kind="ExternalInput")
    z0_ext = nc.dram_tensor("z0", [128, ZP], F32, kind="ExternalInput")
    ct_ext = nc.dram_tensor("ct", [CROWS, NSTEPS * 128], BF16, kind="ExternalInput")
    wt_ext = nc.dram_tensor("wt", [CROWS, HID], BF16, kind="ExternalInput")
    b2c_ext = nc.dram_tensor("b2c", [1, SHARD], BF16, kind="ExternalInput")
    zout_ext = nc.dram_tensor("zout", [128, SHARD], F32, kind="ExternalOutput")

    with tile.TileContext(nc) as tc:
        with (
            tc.tile_pool(name="persist", bufs=1) as persist,
            tc.tile_pool(name="w1", bufs=3) as w1pool,
            tc.tile_pool(name="prez", bufs=2) as przpool,
            tc.tile_pool(name="stage", bufs=1) as stpool,
            tc.tile_pool(name="hback", bufs=2) as hbpool,
            tc.tile_pool(name="h", bufs=2) as hpool,
            tc.tile_pool(name="ps", bufs=8, space="PSUM") as psum,
            tc.tile_pool(name="dram", bufs=2, space="DRAM") as dram,
        ):
            # ---------------- persistent tiles
            w2c_sb = persist.tile([128, NH * SHARD], BF16)   # 138.5KB/part
            z_sb = persist.tile([128, ZP], F32)              # z master, batch layout
            ct_sb = persist.tile([CROWS, NSTEPS * 128], BF16)
            wt_sb = persist.tile([CROWS, HID], BF16)
            b2c_sb = persist.tile([1, SHARD], BF16)
            ones_sb = persist.tile([1, 128], BF16)
            id_bf = persist.tile([128, 128], BF16)
            id_f32 = persist.tile([128, 128], F32)
            # z^T blocks as separate tiles => per-block dependencies, so the
            # next step's mm1 starts as soon as block 0 is refreshed.
            ztb = [persist.tile([128, 128], BF16, name=f"ztb{k}")
                   for k in range(NKZ)]

            nc.gpsimd.memset(ones_sb[:], 1.0)
            masks.make_identity(nc, id_bf[:])
            masks.make_identity(nc, id_f32[:])

            # ---------------- preamble loads
            for k in range(NH):
                nc.sync.dma_start(
                    w2c_sb[:, k * SHARD:(k + 1) * SHARD],
                    w2c_ext[k * 128:(k + 1) * 128, :],
                )
            for k in range(NKZ):
                nc.sync.dma_start(ztb[k][:], z0t_ext[:, k * 128:(k + 1) * 128])
            nc.sync.dma_start(z_sb[:], z0_ext[:])
            nc.sync.dma_start(ct_sb[:], ct_ext[:])
            nc.sync.dma_start(wt_sb[:], wt_ext[:])
            nc.sync.dma_start(b2c_sb[:], b2c_ext[:])

            # ---------------- Euler steps
            for s in range(NSTEPS):

                def mm1_half(hf, s=s):
                    """Partial pre half -> transpose -> bf16 -> AllReduce."""
                    base = hf * HHID
                    pre = [psum.tile([128, 512], F32, tag="ps",
                                     name=f"pre{s}_{hf}_{n}") for n in range(4)]
                    for k in range(NKZ):
                        w1t = w1pool.tile([128, HHID], BF16, tag="w1",
                                          name=f"w1t{s}_{hf}_{k}")
                        nc.sync.dma_start(
                            w1t[:],
                            w1c_ext[k * 128:(k + 1) * 128, base:base + HHID],
                        )
                        for n in range(4):
                            nc.tensor.matmul(
                                pre[n][:],
                                ztb[k][:],
                                w1t[:, n * 512:(n + 1) * 512],
                                start=(k == 0), stop=False,
                            )
                    for n in range(4):
                        nc.tensor.matmul(
                            pre[n][:],
                            ct_sb[:, s * 128:(s + 1) * 128],
                            wt_sb[:, base + n * 512:base + (n + 1) * 512],
                            start=False, stop=True,
                        )
                    # transpose the partial pre on-core (compute-covered):
                    # PSUM -> SBUF f32 -> PE transpose -> bf16 stage (pre^T)
                    st = stpool.tile([128, HHID], BF16, tag="st",
                                     name=f"st{s}_{hf}")
                    for n in range(4):
                        prz = przpool.tile([128, 512], F32, tag="prez",
                                           name=f"prz{s}_{hf}_{n}")
                        nc.vector.tensor_copy(prz[:], pre[n][:])
                        for j in range(4):
                            tp = psum.tile([128, 128], F32, tag="ps",
                                           name=f"tp{s}_{hf}_{n}_{j}")
                            nc.tensor.transpose(
                                tp[:], prz[:, j * 128:(j + 1) * 128], id_f32[:]
                            )
                            blk = 4 * n + j
                            nc.vector.tensor_copy(
                                st[:, blk * 128:(blk + 1) * 128], tp[:]
                            )
                    ccin = dram.tile([128, HHID], BF16, tag="ccin",
                                     name=f"ccin{s}_{hf}")
                    ccout = dram.tile([128, HHID], BF16, tag="ccout",
                                      addr_space="Shared", name=f"ccout{s}_{hf}")
                    nc.sync.dma_start(ccin[:], st[:])
                    nc.gpsimd.collective_compute(
                        "AllReduce",
                        mybir.AluOpType.add,
                        replica_groups=[CORE_IDS],
                        ins=[ccin.opt()],
                        outs=[ccout.opt()],
                    )
                    return ccout

                def tanh_half(hf, ccout, s=s):
                    """DMA summed pre^T back, tanh -> h^T blocks (bf16)."""
                    hb = hbpool.tile([128, HHID], BF16, tag="hb",
                                     name=f"hb{s}_{hf}")
                    hch = hpool.tile([128, HHID], BF16, tag="h",
                                     name=f"h{s}_{hf}")
                    for half in range(2):
                        sl = slice(half * (HHID // 2), (half + 1) * (HHID // 2))
                        nc.sync.dma_start(hb[:, sl], ccout[:, sl])
                        nc.scalar.activation(
                            hch[:, sl], hb[:, sl],
                            mybir.ActivationFunctionType.Tanh,
                        )
                    return hch

                def mm2_half(hf, hch, dz, s=s):
                    """dz += h_half @ W2c[half rows, :]; lhsT = h^T blocks."""
                    for j in range(NHH):
                        k = hf * NHH + j
                        for c, (off, w) in enumerate(DZ_CHUNKS):
                            nc.tensor.matmul(
                                dz[c][:],
                                hch[:, j * 128:(j + 1) * 128],
                                w2c_sb[:, k * SHARD + off:k * SHARD + off + w],
                                start=(k == 0), stop=False,
                            )

                dz = [psum.tile([128, w], F32, tag="ps", name=f"dz{s}_{c}")
                      for c, (_, w) in enumerate(DZ_CHUNKS)]

                cc0 = mm1_half(0)
                cc1 = mm1_half(1)          # overlaps AllReduce of half 0
                h0 = tanh_half(0, cc0)
                mm2_half(0, h0, dz)        # overlaps AllReduce of half 1
                h1 = tanh_half(1, cc1)
                mm2_half(1, h1, dz)

                # bias row: dz += 1 x b2c
                for c, (off, w) in enumerate(DZ_CHUNKS):
                    nc.tensor.matmul(
                        dz[c][:],
                        ones_sb[:],
                        b2c_sb[:, off:off + w],
                        start=False, stop=True,
                    )

                # === z += dz  (f32 master)
                for c, (off, w) in enumerate(DZ_CHUNKS):
                    nc.vector.tensor_add(
                        z_sb[:, off:off + w], z_sb[:, off:off + w], dz[c][:]
                    )

                # === refresh z^T (bf16) for next step's mm1
                if s < NSTEPS - 1:
                    for k in range(NKZ):
                        tp = psum.tile([128, 128], F32, tag="ps",
                                       name=f"zt{s}_{k}")
                        nc.tensor.transpose(
                            tp[:], z_sb[:, k * 128:(k + 1) * 128], id_f32[:]
                        )
                        nc.vector.tensor_copy(ztb[k][:], tp[:])

            # ---------------- output
            nc.sync.dma_start(zout_ext[:], z_sb[:, :SHARD])

    nc.compile()
    return nc


def _get_nc():
    global _CACHED_NC
    if _CACHED_NC is None:
        _CACHED_NC = build_nc()
    return _CACHED_NC


# ---------------------------------------------------------------- host prep
def _time_embeddings(cur_time, tar_time):
    """Replicate reference._time_grid + _time_embedding in numpy (f32)."""
    cur = np.asarray(cur_time, dtype=np.float32)
    tar = np.asarray(tar_time, dtype=np.float32)
    frac = np.linspace(0.0, 1.0, MAX_STEP).astype(np.float32)
    rows = []
    t_cur = cur.copy()
    for j in range(FREQ):
        s = cur[:, j]
        e = tar[:, j]
        wrap = s > e
        e_adj = np.where(wrap, e + PERIODS[j], e)
        inter = s[:, None] + (e_adj - s)[:, None] * frac[None, :]
        inter = np.where(wrap[:, None], np.mod(inter, PERIODS[j]), inter)
        for k in range(MAX_STEP):
            t_cur = t_cur.copy()
            t_cur[:, j] = inter[:, k]
            rows.append(t_cur)
    T = np.stack(rows, axis=0)                      # [12, bs, 3]
    phase = 2.0 * np.pi * (T - INFS[None, None]) / PERIODS[None, None]
    emb = np.concatenate([np.sin(phase), np.cos(phase)], axis=-1)
    return emb.astype(np.float32)                   # [12, bs, 6]


def _assemble_z0(input_freq, input_seq, uid, fuse_embed, n_poi):
    f = np.asarray(input_freq, dtype=np.float32).reshape(BS, -1)     # [128, 3136]
    A = np.concatenate([f, f, f], axis=1)                            # [128, 9408]
    seq = np.asarray(input_seq).astype(np.int64)
    x = np.asarray(fuse_embed, dtype=np.float32)[seq]                # [128, 64, 128]
    x = x.transpose(0, 2, 1).reshape(BS, -1)                         # [128, 8192]
    u = np.asarray(fuse_embed, dtype=np.float32)[
        int(n_poi) + np.asarray(uid).astype(np.int64)
    ].reshape(BS, -1)                                                # [128, 128]
    return np.concatenate([A, x, u], axis=1)                         # [128, 17728]


def kernel(input_freq, input_seq, uid, cur_time, tar_time, fuse_embed,
           W1, b1, W2, b2, n_poi, **_unused):
    nc = _get_nc()

    W1 = np.asarray(W1, dtype=np.float32)
    W2 = np.asarray(W2, dtype=np.float32)
    b1 = np.asarray(b1, dtype=np.float32)
    b2 = np.asarray(b2, dtype=np.float32)

    z0 = _assemble_z0(input_freq, input_seq, uid, fuse_embed, n_poi)
    emb = _time_embeddings(cur_time, tar_time)      # [12, 128, 6]

    # c'_s = [t0e_s, t1e_s, 1] -> transposed [13, 128] per step
    ct = np.empty((CROWS, NSTEPS * 128), dtype=np.float32)
    for s in range(NSTEPS):
        c = np.concatenate(
            [emb[s], emb[s + 1], np.ones((BS, 1), np.float32)], axis=1
        )                                           # [128, 13]
        ct[:, s * 128:(s + 1) * 128] = c.T
    ct_bf = ct.astype(NP_BF16)

    # Wt' = [W1 time-emb rows; b1] / 8  (AllReduce over 8 cores sums it back)
    wt = np.concatenate([W1[Z:], b1[None]], axis=0) / 8.0            # [13, 4096]
    wt_bf = wt.astype(NP_BF16)

    w1_bf = W1[:Z].astype(NP_BF16)                  # [17728, 4096]
    w2_bf = W2.astype(NP_BF16)                      # [4096, 17728]

    in_maps = []
    for c in CORE_IDS:
        lo, hi = c * SHARD, (c + 1) * SHARD
        w1c = np.zeros((ZP, HID), dtype=NP_BF16)
        w1c[:SHARD] = w1_bf[lo:hi]
        w2c = np.ascontiguousarray(w2_bf[:, lo:hi])
        zc = np.zeros((BS, ZP), dtype=np.float32)
        zc[:, :SHARD] = z0[:, lo:hi]
        # z^T blocks: z0t[p, k*128 + b] = zc[b, k*128 + p]
        z0t = np.ascontiguousarray(
            zc.reshape(BS, NKZ, 128).transpose(2, 1, 0).reshape(128, ZP)
        ).astype(NP_BF16)
        b2c = np.ascontiguousarray(b2[lo:hi][None]).astype(NP_BF16)
        in_maps.append({
            "w1c": w1c, "w2c": w2c, "z0t": z0t, "z0": zc,
            "ct": ct_bf, "wt": wt_bf, "b2c": b2c,
        })

    res = run_bass_kernel_spmd(nc, in_maps, CORE_IDS)
    out = np.concatenate(
        [res.results[c]["zout"] for c in CORE_IDS], axis=1
    ).astype(np.float32)
    return out


# revision 16
# speedup vs baseline: 1.1313x; 1.1313x over previous
"""Trainium2 Bass kernel for nn_NeuralODE: 11 Euler steps of
    z += tanh([z, t0e, t1e] @ W1 + b1) @ W2 + b2
with z=[128, 17728], W1=[17740, 4096], W2=[4096, 17728], on 8 NeuronCores.

Strategy: tensor-parallel over the Z dim (17728 = 8 x 2216 per core).
Each core holds z columns [c*2216, (c+1)*2216) and computes a partial
pre-activation  pre_c = z_c @ W1[rows_c, :]  ([128, 4096]), which is
AllReduced across cores (bf16, two 2048-wide halves so the collective
overlaps compute).  The partial pre is TRANSPOSED on-core before the
AllReduce (in the compute-covered window), so the summed result comes back
as pre^T and tanh directly yields h^T blocks — only DMA+tanh sit on the
critical AllReduce -> mm2 path.  dz_c = h @ W2[:, cols_c] is fully local.

Residency: W1's row shard stays RESIDENT in SBUF (so mm1, which covers the
AllReduces, is pure-PE with no HBM traffic competing against the
collective), while W2's column shard STREAMS from HBM each step — its
slabs prefetch during the mm1 phases and are consumed by mm2.  Bounce-
buffer DMAs ride the scalar-engine HWDGE queue so they never queue behind
the W2 slab stream.  Time-embedding + b1 enter as 13 extra contraction
rows (scaled by 1/8 so the AllReduce sums back to x1); b2 enters mm2 as a
K=1 matmul with a ones vector.
"""

import os
import sys

for _p in ("/opt/trn_rl_repo", "/root/.axon_site/_ro/trn_rl_repo"):
    if os.path.isdir(_p) and _p not in sys.path:
        sys.path.append(_p)

import numpy as np

import concourse.bacc as bacc
import concourse.bass as bass
import concourse.mybir as mybir
from concourse import masks, tile
from concourse.bass_utils import run_bass_kernel_spmd

# ---------------------------------------------------------------- constants
N_CORES = 8
CORE_IDS = list(range(N_CORES))
BS = 128                      # batch
H, W, L, D = 56, 56, 64, 128
Z = 3 * H * W + L * D + D     # 17728
HID = 4096
FREQ = 3
MAX_STEP = 4
NSTEPS = MAX_STEP * FREQ - 1  # 11
PERIODS = np.array([24.0, 7.0, 365.0], dtype=np.float32)
INFS = np.array([0.0, 0.0, 0.0], dtype=np.float32)

SHARD = Z // N_CORES          # 2216
NKZ = 18                      # z k-tiles per core (17 full + 1 partial, padded)
ZP = NKZ * 128                # 2304 padded shard
NH = HID // 128               # 32 hid k-tiles
HHID = HID // 2               # 2048: AllReduce half
NHH = HHID // 128             # 16 hid k-tiles per half
# dz N-chunks over the 2216 shard
DZ_CHUNKS = [(0, 512), (512, 512), (1024, 512), (1536, 512), (2048, 168)]
CROWS = 2 * FREQ * 2 + 1      # 13 = t0e(6) + t1e(6) + bias row

F32 = mybir.dt.float32
BF16 = mybir.dt.bfloat16
NP_BF16 = mybir.dt.np(BF16)

_CACHED_NC = None


def build_nc():
    """Build + compile the SPMD program (identical on all 8 cores)."""
    nc = bacc.Bacc(None, num_devices=N_CORES)

    w1c_ext = nc.dram_tensor("w1c", [ZP, HID], BF16, kind="ExternalInput")
    w2c_ext = nc.dram_tensor("w2c", [HID, SHARD], BF16, kind="ExternalInput")
    z0t_ext = nc.dram_tensor("z0t", [128, ZP], BF16, kind="ExternalInput")
    z0_ext = nc.dram_tensor("z0", [128, ZP], F32, kind="ExternalInput")
    ct_ext = nc.dram_tensor("ct", [CROWS, NSTEPS * 128], BF16, kind="ExternalInput")
    wt_ext = nc.dram_tensor("wt", [CROWS, HID], BF16, kind="ExternalInput")
    b2c_ext = nc.dram_tensor("b2c", [1, SHARD], BF16, kind="ExternalInput")
    zout_ext = nc.dram_tensor("zout", [128, SHARD], F32, kind="ExternalOutput")

    with tile.TileContext(nc) as tc:
        with (
            tc.tile_pool(name="persist", bufs=1) as persist,
            tc.tile_pool(name="w2", bufs=5) as w2pool,
            tc.tile_pool(name="prez", bufs=1) as przpool,
            tc.tile_pool(name="stage", bufs=4) as stpool,
            tc.tile_pool(name="hback", bufs=2) as hbpool,
            tc.tile_pool(name="h", bufs=2) as hpool,
            tc.tile_pool(name="ps", bufs=8, space="PSUM") as psum,
            tc.tile_pool(name="dram", bufs=2, space="DRAM") as dram,
        ):
            # ---------------- persistent tiles
            w1c_sb = persist.tile([128, NKZ * HID], BF16)    # 144KB/part resident
            z_sb = persist.tile([128, ZP], F32)              # z master, batch layout
            ct_sb = persist.tile([CROWS, NSTEPS * 128], BF16)
            wt_sb = persist.tile([CROWS, HID], BF16)
            b2c_sb = persist.tile([1, SHARD], BF16)
            ones_sb = persist.tile([1, 128], BF16)
            id_bf = persist.tile([128, 128], BF16)
            id_f32 = persist.tile([128, 128], F32)
            # z^T blocks as separate tiles => per-block dependencies, so the
            # next step's mm1 starts as soon as block 0 is refreshed.
            ztb = [persist.tile([128, 128], BF16, name=f"ztb{k}")
                   for k in range(NKZ)]

            nc.gpsimd.memset(ones_sb[:], 1.0)
            masks.make_identity(nc, id_bf[:])
            masks.make_identity(nc, id_f32[:])

            # ---------------- preamble loads (W1 resident)
            for k in range(NKZ):
                nc.sync.dma_start(
                    w1c_sb[:, k * HID:(k + 1) * HID],
                    w1c_ext[k * 128:(k + 1) * 128, :],
                )
            for k in range(NKZ):
                nc.sync.dma_start(ztb[k][:], z0t_ext[:, k * 128:(k + 1) * 128])
            nc.sync.dma_start(z_sb[:], z0_ext[:])
            nc.sync.dma_start(ct_sb[:], ct_ext[:])
            nc.sync.dma_start(wt_sb[:], wt_ext[:])
            nc.sync.dma_start(b2c_sb[:], b2c_ext[:])

            # ---------------- Euler steps
            for s in range(NSTEPS):

                # W2 slab stream for this step: emitted first so the sync
                # DMA queue prefetches slabs during the (pure-PE) mm1 phases.
                w2sl = []
                for k in range(NH):
                    wt2 = w2pool.tile([128, SHARD], BF16, tag="w2",
                                      name=f"w2t{s}_{k}")
                    nc.sync.dma_start(wt2[:], w2c_ext[k * 128:(k + 1) * 128, :])
                    w2sl.append(wt2)

                def mm1_half(hf, s=s):
                    """Partial pre half -> transpose -> bf16 -> AllReduce."""
                    base = hf * HHID
                    pre = [psum.tile([128, 512], F32, tag="ps",
                                     name=f"pre{s}_{hf}_{n}") for n in range(4)]
                    for k in range(NKZ):
                        for n in range(4):
                            nc.tensor.matmul(
                                pre[n][:],
                                ztb[k][:],
                                w1c_sb[:, k * HID + base + n * 512:
                                       k * HID + base + (n + 1) * 512],
                                start=(k == 0), stop=False,
                            )
                    for n in range(4):
                        nc.tensor.matmul(
                            pre[n][:],
                            ct_sb[:, s * 128:(s + 1) * 128],
                            wt_sb[:, base + n * 512:base + (n + 1) * 512],
                            start=False, stop=True,
                        )
                    # transpose the partial pre on-core (compute-covered):
                    # PSUM -> SBUF bf16 -> PE transpose -> bounce (pre^T)
                    ccin = dram.tile([128, HHID], BF16, tag="ccin",
                                     name=f"ccin{s}_{hf}")
                    ccout = dram.tile([128, HHID], BF16, tag="ccout",
                                      addr_space="Shared", name=f"ccout{s}_{hf}")
                    for n in range(4):
                        prz = przpool.tile([128, 512], BF16, tag="prez",
                                           name=f"prz{s}_{hf}_{n}")
                        nc.vector.tensor_copy(prz[:], pre[n][:])
                        for j in range(4):
                            tp = psum.tile([128, 128], BF16, tag="ps",
                                           name=f"tp{s}_{hf}_{n}_{j}")
                            nc.tensor.transpose(
                                tp[:], prz[:, j * 128:(j + 1) * 128], id_bf[:]
                            )
                            blk = 4 * n + j
                            stt = stpool.tile([128, 128], BF16, tag="st",
                                              name=f"st{s}_{hf}_{blk}")
                            nc.vector.tensor_copy(stt[:], tp[:])
                            nc.scalar.dma_start(
                                ccin[:, blk * 128:(blk + 1) * 128], stt[:]
                            )
                    nc.gpsimd.collective_compute(
                        "AllReduce",
                        mybir.AluOpType.add,
                        replica_groups=[CORE_IDS],
                        ins=[ccin.opt()],
                        outs=[ccout.opt()],
                    )
                    return ccout

                def tanh_half(hf, ccout, s=s):
                    """DMA summed pre^T back, tanh -> h^T blocks (bf16).

                    Chunked 4x so mm2 starts ~1us after the AllReduce lands."""
                    hch = hpool.tile([128, HHID], BF16, tag="h",
                                     name=f"h{s}_{hf}")
                    for q in range(4):
                        sl = slice(q * 512, (q + 1) * 512)
                        hb = hbpool.tile([128, 512], BF16, tag="hb",
                                         name=f"hb{s}_{hf}_{q}")
                        nc.scalar.dma_start(hb[:], ccout[:, sl])
                        nc.scalar.activation(
                            hch[:, sl], hb[:],
                            mybir.ActivationFunctionType.Tanh,
                        )
                    return hch

                def mm2_half(hf, hch, dz, s=s):
                    """dz += h_half @ W2c[half rows, :]; lhsT = h^T blocks."""
                    for j in range(NHH):
                        k = hf * NHH + j
                        for c, (off, w) in enumerate(DZ_CHUNKS):
                            nc.tensor.matmul(
                                dz[c][:],
                                hch[:, j * 128:(j + 1) * 128],
                                w2sl[k][:, off:off + w],
                                start=(k == 0), stop=False,
                            )

                dz = [psum.tile([128, w], F32, tag="ps", name=f"dz{s}_{c}")
                      for c, (_, w) in enumerate(DZ_CHUNKS)]

                cc0 = mm1_half(0)
                cc1 = mm1_half(1)          # overlaps AllReduce of half 0
                h0 = tanh_half(0, cc0)
                mm2_half(0, h0, dz)        # overlaps AllReduce of half 1
                h1 = tanh_half(1, cc1)
                mm2_half(1, h1, dz)

                # bias row: dz += 1 x b2c
                for c, (off, w) in enumerate(DZ_CHUNKS):
                    nc.tensor.matmul(
                        dz[c][:],
                        ones_sb[:],
                        b2c_sb[:, off:off + w],
                        start=False, stop=True,
                    )

                # === z += dz  (f32 master)
                for c, (off, w) in enumerate(DZ_CHUNKS):
                    nc.vector.tensor_add(
                        z_sb[:, off:off + w], z_sb[:, off:off + w], dz[c][:]
                    )

                # === refresh z^T (bf16) for next step's mm1
                if s < NSTEPS - 1:
                    for k in range(NKZ):
                        tp = psum.tile([128, 128], F32, tag="ps",
                                       name=f"zt{s}_{k}")
                        nc.tensor.transpose(
                            tp[:], z_sb[:, k * 128:(k + 1) * 128], id_f32[:]
                        )
                        nc.vector.tensor_copy(ztb[k][:], tp[:])

            # ---------------- output
            nc.sync.dma_start(zout_ext[:], z_sb[:, :SHARD])

    nc.compile()
    return nc


def _get_nc():
    global _CACHED_NC
    if _CACHED_NC is None:
        _CACHED_NC = build_nc()
    return _CACHED_NC


# ---------------------------------------------------------------- host prep
def _time_embeddings(cur_time, tar_time):
    """Replicate reference._time_grid + _time_embedding in numpy (f32)."""
    cur = np.asarray(cur_time, dtype=np.float32)
    tar = np.asarray(tar_time, dtype=np.float32)
    frac = np.linspace(0.0, 1.0, MAX_STEP).astype(np.float32)
    rows = []
    t_cur = cur.copy()
    for j in range(FREQ):
        s = cur[:, j]
        e = tar[:, j]
        wrap = s > e
        e_adj = np.where(wrap, e + PERIODS[j], e)
        inter = s[:, None] + (e_adj - s)[:, None] * frac[None, :]
        inter = np.where(wrap[:, None], np.mod(inter, PERIODS[j]), inter)
        for k in range(MAX_STEP):
            t_cur = t_cur.copy()
            t_cur[:, j] = inter[:, k]
            rows.append(t_cur)
    T = np.stack(rows, axis=0)                      # [12, bs, 3]
    phase = 2.0 * np.pi * (T - INFS[None, None]) / PERIODS[None, None]
    emb = np.concatenate([np.sin(phase), np.cos(phase)], axis=-1)
    return emb.astype(np.float32)                   # [12, bs, 6]


def _assemble_z0(input_freq, input_seq, uid, fuse_embed, n_poi):
    f = np.asarray(input_freq, dtype=np.float32).reshape(BS, -1)     # [128, 3136]
    A = np.concatenate([f, f, f], axis=1)                            # [128, 9408]
    seq = np.asarray(input_seq).astype(np.int64)
    x = np.asarray(fuse_embed, dtype=np.float32)[seq]                # [128, 64, 128]
    x = x.transpose(0, 2, 1).reshape(BS, -1)                         # [128, 8192]
    u = np.asarray(fuse_embed, dtype=np.float32)[
        int(n_poi) + np.asarray(uid).astype(np.int64)
    ].reshape(BS, -1)                                                # [128, 128]
    return np.concatenate([A, x, u], axis=1)                         # [128, 17728]


def kernel(input_freq, input_seq, uid, cur_time, tar_time, fuse_embed,
           W1, b1, W2, b2, n_poi, **_unused):
    nc = _get_nc()

    W1 = np.asarray(W1, dtype=np.float32)
    W2 = np.asarray(W2, dtype=np.float32)
    b1 = np.asarray(b1, dtype=np.float32)
    b2 = np.asarray(b2, dtype=np.float32)

    z0 = _assemble_z0(input_freq, input_seq, uid, fuse_embed, n_poi)
    emb = _time_embeddings(cur_time, tar_time)      # [12, 128, 6]

    # c'_s = [t0e_s, t1e_s, 1] -> transposed [13, 128] per step
    ct = np.empty((CROWS, NSTEPS * 128), dtype=np.float32)
    for s in range(NSTEPS):
        c = np.concatenate(
            [emb[s], emb[s + 1], np.ones((BS, 1), np.float32)], axis=1
        )                                           # [128, 13]
        ct[:, s * 128:(s + 1) * 128] = c.T
    ct_bf = ct.astype(NP_BF16)

    # Wt' = [W1 time-emb rows; b1] / 8  (AllReduce over 8 cores sums it back)
    wt = np.concatenate([W1[Z:], b1[None]], axis=0) / 8.0            # [13, 4096]
    wt_bf = wt.astype(NP_BF16)

    w1_bf = W1[:Z].astype(NP_BF16)                  # [17728, 4096]
    w2_bf = W2.astype(NP_BF16)                      # [4096, 17728]

    in_maps = []
    for c in CORE_IDS:
        lo, hi = c * SHARD, (c + 1) * SHARD
        w1c = np.zeros((ZP, HID), dtype=NP_BF16)
        w1c[:SHARD] = w1_bf[lo:hi]
        w2c = np.ascontiguousarray(w2_bf[:, lo:hi])
        zc = np.zeros((BS, ZP), dtype=np.float32)
        zc[:, :SHARD] = z0[:, lo:hi]
        # z^T blocks: z0t[p, k*128 + b] = zc[b, k*128 + p]
        z0t = np.ascontiguousarray(
            zc.reshape(BS, NKZ, 128).transpose(2, 1, 0).reshape(128, ZP)
        ).astype(NP_BF16)
        b2c = np.ascontiguousarray(b2[lo:hi][None]).astype(NP_BF16)
        in_maps.append({
            "w1c": w1c, "w2c": w2c, "z0t": z0t, "z0": zc,
            "ct": ct_bf, "wt": wt_bf, "b2c": b2c,
        })

    res = run_bass_kernel_spmd(nc, in_maps, CORE_IDS)
    out = np.concatenate(
        [res.results[c]["zout"] for c in CORE_IDS], axis=1
    ).astype(np.float32)
    return out


# revision 17
# speedup vs baseline: 1.1859x; 1.0483x over previous
"""Trainium2 Bass kernel for nn_NeuralODE: 11 Euler steps of
    z += tanh([z, t0e, t1e] @ W1 + b1) @ W2 + b2
with z=[128, 17728], W1=[17740, 4096], W2=[4096, 17728], on 8 NeuronCores.

Strategy: tensor-parallel over the Z dim (17728 = 8 x 2216 per core).
Each core holds z columns [c*2216, (c+1)*2216) and computes a partial
pre-activation  pre_c = z_c @ W1[rows_c, :]  ([128, 4096]), which is
AllReduced across cores (bf16, two 2048-wide halves so the collective
overlaps compute).  The partial pre is TRANSPOSED on-core before the
AllReduce (in the compute-covered window), so the summed result comes back
as pre^T and tanh directly yields h^T blocks — nothing but DMA+tanh sits on
the critical AllReduce -> mm2 path.  dz_c = h @ W2[:, cols_c] is fully
local.  W2's column shard stays resident in SBUF (bf16); W1's row shard
streams from HBM every step.  Time-embedding + b1 enter as 13 extra
contraction rows (scaled by 1/8 so the AllReduce sums back to x1); b2
enters mm2 as a K=1 matmul with a ones vector.
"""

import os
import sys

for _p in ("/opt/trn_rl_repo", "/root/.axon_site/_ro/trn_rl_repo"):
    if os.path.isdir(_p) and _p not in sys.path:
        sys.path.append(_p)

import numpy as np

import concourse.bacc as bacc
import concourse.bass as bass
import concourse.mybir as mybir
from concourse import masks, tile
from concourse.bass_utils import run_bass_kernel_spmd

# ---------------------------------------------------------------- constants
N_CORES = 8
CORE_IDS = list(range(N_CORES))
BS = 128                      # batch
H, W, L, D = 56, 56, 64, 128
Z = 3 * H * W + L * D + D     # 17728
HID = 4096
FREQ = 3
MAX_STEP = 4
NSTEPS = MAX_STEP * FREQ - 1  # 11
PERIODS = np.array([24.0, 7.0, 365.0], dtype=np.float32)
INFS = np.array([0.0, 0.0, 0.0], dtype=np.float32)

SHARD = Z // N_CORES          # 2216
NKZ = 18                      # z k-tiles per core (17 full + 1 partial, padded)
ZP = NKZ * 128                # 2304 padded shard
NH = HID // 128               # 32 hid k-tiles
HHID = HID // 2               # 2048: AllReduce half
NHH = HHID // 128             # 16 hid k-tiles per half
# dz N-chunks over the 2216 shard
DZ_CHUNKS = [(0, 512), (512, 512), (1024, 512), (1536, 512), (2048, 168)]
CROWS = 2 * FREQ * 2 + 1      # 13 = t0e(6) + t1e(6) + bias row

F32 = mybir.dt.float32
BF16 = mybir.dt.bfloat16
NP_BF16 = mybir.dt.np(BF16)

_CACHED_NC = None


def build_nc():
    """Build + compile the SPMD program (identical on all 8 cores)."""
    nc = bacc.Bacc(None, num_devices=N_CORES)

    w1c_ext = nc.dram_tensor("w1c", [2, ZP, HHID], BF16, kind="ExternalInput")
    w2c_ext = nc.dram_tensor("w2c", [HID, SHARD], BF16, kind="ExternalInput")
    z0t_ext = nc.dram_tensor("z0t", [128, ZP], BF16, kind="ExternalInput")
    z0_ext = nc.dram_tensor("z0", [128, ZP], F32, kind="ExternalInput")
    ct_ext = nc.dram_tensor("ct", [CROWS, NSTEPS * 128], BF16, kind="ExternalInput")
    wt_ext = nc.dram_tensor("wt", [CROWS, HID], BF16, kind="ExternalInput")
    b2c_ext = nc.dram_tensor("b2c", [1, SHARD], BF16, kind="ExternalInput")
    zout_ext = nc.dram_tensor("zout", [128, SHARD], F32, kind="ExternalOutput")

    with tile.TileContext(nc) as tc:
        with (
            tc.tile_pool(name="persist", bufs=1) as persist,
            tc.tile_pool(name="w1", bufs=4) as w1pool,
            tc.tile_pool(name="prez", bufs=2) as przpool,
            tc.tile_pool(name="stage", bufs=1) as stpool,
            tc.tile_pool(name="hback", bufs=2) as hbpool,
            tc.tile_pool(name="h", bufs=2) as hpool,
            tc.tile_pool(name="ps", bufs=8, space="PSUM") as psum,
            tc.tile_pool(name="dram", bufs=2, space="DRAM") as dram,
        ):
            # ---------------- persistent tiles
            w2c_sb = persist.tile([128, NH * SHARD], BF16)   # 138.5KB/part
            z_sb = persist.tile([128, ZP], F32)              # z master, batch layout
            ct_sb = persist.tile([CROWS, NSTEPS * 128], BF16)
            wt_sb = persist.tile([CROWS, HID], BF16)
            b2c_sb = persist.tile([1, SHARD], BF16)
            ones_sb = persist.tile([1, 128], BF16)
            id_bf = persist.tile([128, 128], BF16)
            id_f32 = persist.tile([128, 128], F32)
            # z^T blocks as separate tiles => per-block dependencies, so the
            # next step's mm1 starts as soon as block 0 is refreshed.
            ztb = [persist.tile([128, 128], BF16, name=f"ztb{k}")
                   for k in range(NKZ)]

            nc.gpsimd.memset(ones_sb[:], 1.0)
            masks.make_identity(nc, id_bf[:])
            masks.make_identity(nc, id_f32[:])

            # ---------------- preamble loads
            for k in range(NH):
                nc.sync.dma_start(
                    w2c_sb[:, k * SHARD:(k + 1) * SHARD],
                    w2c_ext[k * 128:(k + 1) * 128, :],
                )
            for k in range(NKZ):
                nc.sync.dma_start(ztb[k][:], z0t_ext[:, k * 128:(k + 1) * 128])
            nc.sync.dma_start(z_sb[:], z0_ext[:])
            nc.sync.dma_start(ct_sb[:], ct_ext[:])
            nc.sync.dma_start(wt_sb[:], wt_ext[:])
            nc.sync.dma_start(b2c_sb[:], b2c_ext[:])

            # ---------------- Euler steps
            for s in range(NSTEPS):

                def mm1_half(hf, s=s):
                    """Partial pre half -> transpose -> bf16 -> AllReduce."""
                    base = hf * HHID
                    pre = [psum.tile([128, 512], F32, tag="ps",
                                     name=f"pre{s}_{hf}_{n}") for n in range(4)]
                    for k in range(NKZ):
                        w1t = w1pool.tile([128, HHID], BF16, tag="w1",
                                          name=f"w1t{s}_{hf}_{k}")
                        nc.sync.dma_start(
                            w1t[:], w1c_ext[hf, k * 128:(k + 1) * 128, :]
                        )
                        for n in range(4):
                            nc.tensor.matmul(
                                pre[n][:],
                                ztb[k][:],
                                w1t[:, n * 512:(n + 1) * 512],
                                start=(k == 0), stop=False,
                            )
                    for n in range(4):
                        nc.tensor.matmul(
                            pre[n][:],
                            ct_sb[:, s * 128:(s + 1) * 128],
                            wt_sb[:, base + n * 512:base + (n + 1) * 512],
                            start=False, stop=True,
                        )
                    # transpose the partial pre on-core (compute-covered):
                    # PSUM -> SBUF f32 -> PE transpose -> bf16 stage (pre^T)
                    st = stpool.tile([128, HHID], BF16, tag="st",
                                     name=f"st{s}_{hf}")
                    for n in range(4):
                        prz = przpool.tile([128, 512], BF16, tag="prez",
                                           name=f"prz{s}_{hf}_{n}")
                        nc.vector.tensor_copy(prz[:], pre[n][:])
                        for j in range(4):
                            tp = psum.tile([128, 128], BF16, tag="ps",
                                           name=f"tp{s}_{hf}_{n}_{j}")
                            nc.tensor.transpose(
                                tp[:], prz[:, j * 128:(j + 1) * 128], id_bf[:]
                            )
                            blk = 4 * n + j
                            nc.vector.tensor_copy(
                                st[:, blk * 128:(blk + 1) * 128], tp[:]
                            )
                    ccin = dram.tile([128, HHID], BF16, tag="ccin",
                                     name=f"ccin{s}_{hf}")
                    ccout = dram.tile([128, HHID], BF16, tag="ccout",
                                      addr_space="Shared", name=f"ccout{s}_{hf}")
                    nc.scalar.dma_start(ccin[:], st[:])
                    nc.gpsimd.collective_compute(
                        "AllReduce",
                        mybir.AluOpType.add,
                        replica_groups=[CORE_IDS],
                        ins=[ccin.opt()],
                        outs=[ccout.opt()],
                    )
                    return ccout

                def tanh_half(hf, ccout, s=s):
                    """DMA summed pre^T back, tanh -> h^T blocks (bf16)."""
                    hb = hbpool.tile([128, HHID], BF16, tag="hb",
                                     name=f"hb{s}_{hf}")
                    hch = hpool.tile([128, HHID], BF16, tag="h",
                                     name=f"h{s}_{hf}")
                    for half in range(2):
                        sl = slice(half * (HHID // 2), (half + 1) * (HHID // 2))
                        nc.scalar.dma_start(hb[:, sl], ccout[:, sl])
                        nc.scalar.activation(
                            hch[:, sl], hb[:, sl],
                            mybir.ActivationFunctionType.Tanh,
                        )
                    return hch

                def mm2_half(hf, hch, dz, s=s):
                    """dz += h_half @ W2c[half rows, :]; lhsT = h^T blocks."""
                    for j in range(NHH):
                        k = hf * NHH + j
                        for c, (off, w) in enumerate(DZ_CHUNKS):
                            nc.tensor.matmul(
                                dz[c][:],
                                hch[:, j * 128:(j + 1) * 128],
                                w2c_sb[:, k * SHARD + off:k * SHARD + off + w],
                                start=(k == 0), stop=False,
                            )

                dz = [psum.tile([128, w], F32, tag="ps", name=f"dz{s}_{c}")
                      for c, (_, w) in enumerate(DZ_CHUNKS)]

                cc0 = mm1_half(0)
                cc1 = mm1_half(1)          # overlaps AllReduce of half 0
                h0 = tanh_half(0, cc0)
                mm2_half(0, h0, dz)        # overlaps AllReduce of half 1
                h1 = tanh_half(1, cc1)
                mm2_half(1, h1, dz)

                # bias row: dz += 1 x b2c
                for c, (off, w) in enumerate(DZ_CHUNKS):
                    nc.tensor.matmul(
                        dz[c][:],
                        ones_sb[:],
                        b2c_sb[:, off:off + w],
                        start=False, stop=True,
                    )

                # === z += dz  (f32 master)
                for c, (off, w) in enumerate(DZ_CHUNKS):
                    nc.vector.tensor_add(
                        z_sb[:, off:off + w], z_sb[:, off:off + w], dz[c][:]
                    )

                # === refresh z^T (bf16) for next step's mm1
                if s < NSTEPS - 1:
                    for k in range(NKZ):
                        tp = psum.tile([128, 128], F32, tag="ps",
                                       name=f"zt{s}_{k}")
                        nc.tensor.transpose(
                            tp[:], z_sb[:, k * 128:(k + 1) * 128], id_f32[:]
                        )
                        nc.vector.tensor_copy(ztb[k][:], tp[:])

            # ---------------- output
            nc.sync.dma_start(zout_ext[:], z_sb[:, :SHARD])

    nc.compile()
    return nc


def _get_nc():
    global _CACHED_NC
    if _CACHED_NC is None:
        _CACHED_NC = build_nc()
    return _CACHED_NC


# ---------------------------------------------------------------- host prep
def _time_embeddings(cur_time, tar_time):
    """Replicate reference._time_grid + _time_embedding in numpy (f32)."""
    cur = np.asarray(cur_time, dtype=np.float32)
    tar = np.asarray(tar_time, dtype=np.float32)
    frac = np.linspace(0.0, 1.0, MAX_STEP).astype(np.float32)
    rows = []
    t_cur = cur.copy()
    for j in range(FREQ):
        s = cur[:, j]
        e = tar[:, j]
        wrap = s > e
        e_adj = np.where(wrap, e + PERIODS[j], e)
        inter = s[:, None] + (e_adj - s)[:, None] * frac[None, :]
        inter = np.where(wrap[:, None], np.mod(inter, PERIODS[j]), inter)
        for k in range(MAX_STEP):
            t_cur = t_cur.copy()
            t_cur[:, j] = inter[:, k]
            rows.append(t_cur)
    T = np.stack(rows, axis=0)                      # [12, bs, 3]
    phase = 2.0 * np.pi * (T - INFS[None, None]) / PERIODS[None, None]
    emb = np.concatenate([np.sin(phase), np.cos(phase)], axis=-1)
    return emb.astype(np.float32)                   # [12, bs, 6]


def _assemble_z0(input_freq, input_seq, uid, fuse_embed, n_poi):
    f = np.asarray(input_freq, dtype=np.float32).reshape(BS, -1)     # [128, 3136]
    A = np.concatenate([f, f, f], axis=1)                            # [128, 9408]
    seq = np.asarray(input_seq).astype(np.int64)
    x = np.asarray(fuse_embed, dtype=np.float32)[seq]                # [128, 64, 128]
    x = x.transpose(0, 2, 1).reshape(BS, -1)                         # [128, 8192]
    u = np.asarray(fuse_embed, dtype=np.float32)[
        int(n_poi) + np.asarray(uid).astype(np.int64)
    ].reshape(BS, -1)                                                # [128, 128]
    return np.concatenate([A, x, u], axis=1)                         # [128, 17728]


def kernel(input_freq, input_seq, uid, cur_time, tar_time, fuse_embed,
           W1, b1, W2, b2, n_poi, **_unused):
    nc = _get_nc()

    W1 = np.asarray(W1, dtype=np.float32)
    W2 = np.asarray(W2, dtype=np.float32)
    b1 = np.asarray(b1, dtype=np.float32)
    b2 = np.asarray(b2, dtype=np.float32)

    z0 = _assemble_z0(input_freq, input_seq, uid, fuse_embed, n_poi)
    emb = _time_embeddings(cur_time, tar_time)      # [12, 128, 6]

    # c'_s = [t0e_s, t1e_s, 1] -> transposed [13, 128] per step
    ct = np.empty((CROWS, NSTEPS * 128), dtype=np.float32)
    for s in range(NSTEPS):
        c = np.concatenate(
            [emb[s], emb[s + 1], np.ones((BS, 1), np.float32)], axis=1
        )                                           # [128, 13]
        ct[:, s * 128:(s + 1) * 128] = c.T
    ct_bf = ct.astype(NP_BF16)

    # Wt' = [W1 time-emb rows; b1] / 8  (AllReduce over 8 cores sums it back)
    wt = np.concatenate([W1[Z:], b1[None]], axis=0) / 8.0            # [13, 4096]
    wt_bf = wt.astype(NP_BF16)

    w1_bf = W1[:Z].astype(NP_BF16)                  # [17728, 4096]
    w2_bf = W2.astype(NP_BF16)                      # [4096, 17728]

    in_maps = []
    for c in CORE_IDS:
        lo, hi = c * SHARD, (c + 1) * SHARD
        w1c = np.zeros((2, ZP, HHID), dtype=NP_BF16)
        w1c[0, :SHARD] = w1_bf[lo:hi, :HHID]
        w1c[1, :SHARD] = w1_bf[lo:hi, HHID:]
        w2c = np.ascontiguousarray(w2_bf[:, lo:hi])
        zc = np.zeros((BS, ZP), dtype=np.float32)
        zc[:, :SHARD] = z0[:, lo:hi]
        # z^T blocks: z0t[p, k*128 + b] = zc[b, k*128 + p]
        z0t = np.ascontiguousarray(
            zc.reshape(BS, NKZ, 128).transpose(2, 1, 0).reshape(128, ZP)
        ).astype(NP_BF16)
        b2c = np.ascontiguousarray(b2[lo:hi][None]).astype(NP_BF16)
        in_maps.append({
            "w1c": w1c, "w2c": w2c, "z0t": z0t, "z0": zc,
            "ct": ct_bf, "wt": wt_bf, "b2c": b2c,
        })

    res = run_bass_kernel_spmd(nc, in_maps, CORE_IDS)
    out = np.concatenate(
        [res.results[c]["zout"] for c in CORE_IDS], axis=1
    ).astype(np.float32)
    return out
